# revision 14
# baseline (speedup 1.0000x reference)
"""1-D nearest-neighbor retrieval kernel for Trainium2 (8 NeuronCores).

For each query x[b], finds argmin_n |input_tensor[n] - x[b]| and returns
accuracy_tensor[argmin].  Queries are sharded across the 8 cores (512 each,
4 query tiles of 128 partitions); the index tables are replicated.

Index-based algorithm (host builds a sorted index; device does O(NSEG)
work per query instead of O(N)):
  Host prep: stable-argsort the refs; build
    - bnd[NSEG]: first sorted ref of each 128-wide segment,
    - tab[NSEG, RPW]: per segment a window of the sorted refs (LPAD/RPAD
      overlap into neighboring segments for duplicate chains that
      straddle a boundary), the matching accuracy values, and the
      matching ORIGINAL indices (fp32-exact, < 2^16).
  Device, pipelined per query tile [128 queries in partitions]:
    1. seg = clamp(#{bnd_s <= x} - 1, 0): fused is_le+sum on DVE
       (tensor_scalar with accum_out) for two query tiles, and
       Sign(Relu(bnd-x)) with fused sum on ScalarE for the other two;
       the boundary table is broadcast by two parallel half-DMAs
       (Scalar HW-DGE + GpSimd SWDGE queues).
    2. One indirect-DMA row gather per query tile, issued as soon as its
       own seg is ready (GpSimd SWDGE queue pipelines the four rows).
    3. dist = |ref - x| via ScalarE Abs with bias=-x (bit-identical to
       the fp32 reference); per-tile DVE chain: gmin = min(dist);
       tie-break exactly like jnp.argmin (lowest ORIGINAL index) via
       cand = idx - 2^17 * (dist == gmin); om = min(cand); answer =
       sum(acc * (cand == om)).  All index arithmetic is exact in fp32.

Monotonicity of fp32 rounding guarantees the global argmin lives in the
gathered window: correctly-rounded subtract is monotone, and near-ties
from rounding cannot occur because nearest-neighbor subtractions are
exact (Sterbenz).  Duplicate values are covered by the stable sort plus
the LPAD/RPAD overlap; exact midpoint ties are covered by the original-
index tie-break.  Bit-for-bit identical to the jax reference.
"""
from contextlib import ExitStack

import numpy as np

import concourse.bass as bass
import concourse.bacc as bacc
import concourse.tile as tile
from concourse import mybir
from concourse._compat import with_exitstack
from concourse.bass_utils import run_bass_kernel_spmd

P = 128
N_CORES = 8
B = 4096
B_CORE = B // N_CORES  # 512
N = 65536
W = 128                 # segment width
NSEG = N // W           # 512 segments
LPAD = 4                # window overlap for duplicate chains (max dup run 3)
RPAD = 4
ROWL = W + LPAD + RPAD  # 136
RPW = 416               # row: refs[0:136] | acc[136:272] | idx[272:408] | pad
N_QT = B_CORE // P      # 4 query tiles per core
SKEW = -131072.0        # -2^17: keeps idx-skew arithmetic exact in fp32

FP32 = mybir.dt.float32
U32 = mybir.dt.uint32


@with_exitstack
def _nn_kernel(ctx: ExitStack, tc: tile.TileContext, xq, bnd, tab, out):
    nc = tc.nc

    persist = ctx.enter_context(tc.tile_pool(name="persist", bufs=1))

    # Constants + a dummy activation that pulls the ~1.3us ACT table load
    # off the critical path before the input DMAs complete.
    bm1 = persist.tile([P, 1], FP32, tag="bm1")
    nc.gpsimd.memset(bm1[:], -1.0)
    b511 = persist.tile([P, 1], FP32, tag="b511")
    nc.gpsimd.memset(b511[:], 511.0)

    # Input DMAs first (Scalar HW-DGE queue; the Sync queue lowers
    # broadcasts into pathological 112-byte packets whose ring drain
    # stalls the NEFF teardown barrier by ~10us).  The bnd broadcast is
    # split with the GpSimd SWDGE queue, which is idle this early.
    x_sb = persist.tile([P, N_QT], FP32, tag="x_sb")
    nc.scalar.dma_start(out=x_sb[:], in_=xq.rearrange("(p q) -> p q", p=P))
    H = NSEG // 2
    bnd_b = persist.tile([P, NSEG], FP32, tag="bnd_b")
    nc.scalar.dma_start(
        out=bnd_b[:, 0:H], in_=bnd[None, 0:H].to_broadcast([P, H])
    )
    nc.gpsimd.dma_start(
        out=bnd_b[:, H:NSEG], in_=bnd[None, H:NSEG].to_broadcast([P, H])
    )
    warm = persist.tile([P, 1], FP32, tag="warm")
    nc.scalar.activation(warm[:], bm1[:], mybir.ActivationFunctionType.Relu)
    neg_x = persist.tile([P, N_QT], FP32, tag="neg_x")
    nc.vector.tensor_scalar_mul(neg_x[:], x_sb[:], -1.0)

    cmp = persist.tile([P, NSEG], FP32, tag="cmp")
    relu_t = persist.tile([P, NSEG], FP32, tag="relu_t")
    sign_t = persist.tile([P, NSEG], FP32, tag="sign_t")
    cnt4 = persist.tile([P, N_QT], FP32, tag="cnt4")
    seg4 = persist.tile([P, N_QT], FP32, tag="seg4")
    seg_u4 = persist.tile([P, N_QT], U32, tag="seg_u4")
    gra4 = persist.tile([P, N_QT * RPW], FP32, tag="gra4")
    dist4 = persist.tile([P, N_QT * ROWL], FP32, tag="dist4")

    # --- per query tile: cnt -> seg, split across ScalarE and DVE ---
    # ACT path (slots 0,1): p = sum(Sign(Relu(bnd - x))) = #{bnd > x};
    #   seg = Relu(511 - p) = clamp(#{bnd <= x} - 1, 0).
    # DVE path (slots 2,3): fused is_le+sum, then seg = Relu(cnt - 1).
    for qt in (0, 1):
        nc.scalar.activation(
            relu_t[:],
            bnd_b[:],
            mybir.ActivationFunctionType.Relu,
            bias=neg_x[:, qt : qt + 1],
            scale=1.0,
        )
        nc.scalar.activation(
            sign_t[:],
            relu_t[:],
            mybir.ActivationFunctionType.Sign,
            accum_out=cnt4[:, qt : qt + 1],
        )
        nc.scalar.activation(
            seg4[:, qt : qt + 1],
            cnt4[:, qt : qt + 1],
            mybir.ActivationFunctionType.Relu,
            bias=b511[:, 0:1],
            scale=-1.0,
        )
    for qt in (2, 3):
        nc.vector.tensor_scalar(
            cmp[:],
            bnd_b[:],
            x_sb[:, qt : qt + 1],
            None,
            op0=mybir.AluOpType.is_le,
            op1=mybir.AluOpType.add,
            accum_out=cnt4[:, qt : qt + 1],
        )
        nc.scalar.activation(
            seg4[:, qt : qt + 1],
            cnt4[:, qt : qt + 1],
            mybir.ActivationFunctionType.Relu,
            bias=bm1[:, 0:1],
            scale=1.0,
        )
    for qt in range(N_QT):
        nc.vector.tensor_copy(seg_u4[:, qt : qt + 1], seg4[:, qt : qt + 1])
        nc.gpsimd.indirect_dma_start(
            out=gra4[:, qt * RPW : (qt + 1) * RPW],
            out_offset=None,
            in_=tab,
            in_offset=bass.IndirectOffsetOnAxis(ap=seg_u4[:, qt : qt + 1], axis=0),
        )

    # --- phase 2 in slot pairs: |ref-x|, argmin, orig-index tie-break ---
    stage = persist.tile([P, N_QT], FP32, tag="stage")
    for pr in range(N_QT // 2):
        q0 = 2 * pr
        L = 2 * ROWL
        for qt in (q0, q0 + 1):
            nc.scalar.activation(
                dist4[:, qt * ROWL : (qt + 1) * ROWL],
                gra4[:, qt * RPW : qt * RPW + ROWL],
                mybir.ActivationFunctionType.Abs,
                bias=neg_x[:, qt : qt + 1],
                scale=1.0,
            )
        dist2 = dist4[:, q0 * ROWL : q0 * ROWL + L]
        dist2_v = dist2.rearrange("p (q r) -> p q r", q=2)
        gpair_v = gra4[:, q0 * RPW : (q0 + 2) * RPW].rearrange(
            "p (q r) -> p q r", q=2
        )
        gmin2 = persist.tile([P, 2], FP32, tag=f"gmin{pr}")
        nc.vector.tensor_reduce(
            gmin2[:], dist2_v, axis=mybir.AxisListType.X, op=mybir.AluOpType.min
        )
        mask2 = persist.tile([P, L], FP32, tag=f"mask{pr}")
        nc.vector.tensor_tensor(
            out=mask2[:].rearrange("p (q r) -> p q r", q=2),
            in0=dist2_v,
            in1=gmin2[:]
            .rearrange("p (q o) -> p q o", o=1)
            .to_broadcast([P, 2, ROWL]),
            op=mybir.AluOpType.is_equal,
        )
        skew2 = persist.tile([P, L], FP32, tag=f"skew{pr}")
        nc.scalar.activation(
            skew2[:],
            mask2[:],
            mybir.ActivationFunctionType.Copy,
            scale=SKEW,
        )
        cand2 = persist.tile([P, L], FP32, tag=f"cand{pr}")
        nc.vector.tensor_tensor(
            out=cand2[:].rearrange("p (q r) -> p q r", q=2),
            in0=skew2[:].rearrange("p (q r) -> p q r", q=2),
            in1=gpair_v[:, :, 2 * ROWL : 3 * ROWL],
            op=mybir.AluOpType.add,
        )
        om2 = persist.tile([P, 2], FP32, tag=f"om{pr}")
        nc.vector.tensor_reduce(
            om2[:],
            cand2[:].rearrange("p (q r) -> p q r", q=2),
            axis=mybir.AxisListType.X,
            op=mybir.AluOpType.min,
        )
        oh2 = persist.tile([P, L], FP32, tag=f"oh{pr}")
        nc.vector.tensor_tensor(
            out=oh2[:].rearrange("p (q r) -> p q r", q=2),
            in0=cand2[:].rearrange("p (q r) -> p q r", q=2),
            in1=om2[:]
            .rearrange("p (q o) -> p q o", o=1)
            .to_broadcast([P, 2, ROWL]),
            op=mybir.AluOpType.is_equal,
        )
        sel2 = persist.tile([P, L], FP32, tag=f"sel{pr}")
        nc.vector.tensor_tensor(
            out=sel2[:].rearrange("p (q r) -> p q r", q=2),
            in0=oh2[:].rearrange("p (q r) -> p q r", q=2),
            in1=gpair_v[:, :, ROWL : 2 * ROWL],
            op=mybir.AluOpType.mult,
        )
        nc.vector.tensor_reduce(
            stage[:, q0 : q0 + 2],
            sel2[:].rearrange("p (q r) -> p q r", q=2),
            axis=mybir.AxisListType.X,
            op=mybir.AluOpType.add,
        )
    nc.scalar.dma_start(out=out.rearrange("(p q) -> p q", p=P), in_=stage[:])


_CACHED_NC = None


def _build():
    global _CACHED_NC
    if _CACHED_NC is not None:
        return _CACHED_NC
    nc = bacc.Bacc("TRN2", target_bir_lowering=False, debug=False)
    xq = nc.dram_tensor("xq", [B_CORE], FP32, kind="ExternalInput").ap()
    bnd = nc.dram_tensor("bnd", [NSEG], FP32, kind="ExternalInput").ap()
    tab = nc.dram_tensor("tab", [NSEG, RPW], FP32, kind="ExternalInput").ap()
    out = nc.dram_tensor("out", [B_CORE], FP32, kind="ExternalOutput").ap()
    with tile.TileContext(nc) as tc:
        _nn_kernel(tc, xq, bnd, tab, out)
    nc.compile()
    _CACHED_NC = nc
    return nc


def _build_tables(refs, acc):
    order = np.argsort(refs, kind="stable")
    sref = refs[order]
    sacc = acc[order]
    sidx = order.astype(np.float32)
    bnd = np.ascontiguousarray(sref[0::W])
    sref_p = np.concatenate(
        [np.full(LPAD, -1e30, np.float32), sref, np.full(RPAD, 1e30, np.float32)]
    )
    sacc_p = np.concatenate(
        [np.zeros(LPAD, np.float32), sacc, np.zeros(RPAD, np.float32)]
    )
    sidx_p = np.concatenate(
        [np.zeros(LPAD, np.float32), sidx, np.zeros(RPAD, np.float32)]
    )
    gi = (np.arange(NSEG) * W)[:, None] + np.arange(ROWL)[None, :]
    tab = np.zeros((NSEG, RPW), np.float32)
    tab[:, 0:ROWL] = sref_p[gi]
    tab[:, ROWL : 2 * ROWL] = sacc_p[gi]
    tab[:, 2 * ROWL : 3 * ROWL] = sidx_p[gi]
    return bnd, np.ascontiguousarray(tab)


def kernel(x, input_tensor, accuracy_tensor):
    x = np.asarray(x, dtype=np.float32)
    refs = np.asarray(input_tensor, dtype=np.float32)
    acc = np.asarray(accuracy_tensor, dtype=np.float32)

    bnd, tab = _build_tables(refs, acc)
    nc = _build()
    in_maps = [
        {
            "xq": np.ascontiguousarray(x[i * B_CORE : (i + 1) * B_CORE]),
            "bnd": bnd,
            "tab": tab,
        }
        for i in range(N_CORES)
    ]
    res = run_bass_kernel_spmd(nc, in_maps, core_ids=list(range(N_CORES)))
    return np.concatenate([res.results[i]["out"] for i in range(N_CORES)])


# revision 15
# speedup vs baseline: 1.1389x; 1.1389x over previous
"""1-D nearest-neighbor retrieval kernel for Trainium2 (8 NeuronCores).

For each query x[b], finds argmin_n |input_tensor[n] - x[b]| and returns
accuracy_tensor[argmin].  Queries are sharded across the 8 cores (512 each,
4 query tiles of 128 partitions); the index tables are replicated.

Index-based algorithm (host builds a sorted index; device does O(NSEG)
work per query instead of O(N)):
  Host prep: stable-argsort the refs; build
    - bnd[NSEG]: first sorted ref of each 128-wide segment,
    - tab[NSEG, RPW]: per segment a window of the sorted refs (LPAD/RPAD
      overlap into neighboring segments for duplicate chains that
      straddle a boundary), the matching accuracy values, and the
      matching ORIGINAL indices (fp32-exact, < 2^16).
  Device, pipelined per query tile [128 queries in partitions]:
    1. seg = clamp(#{bnd_s <= x} - 1, 0): fused is_le+sum on DVE
       (tensor_scalar with accum_out), Relu(cnt-1) on ScalarE; the
       boundary table is broadcast on the Scalar HW-DGE queue.
    2. One indirect-DMA row gather per query tile, issued as soon as its
       own seg is ready (GpSimd SWDGE queue pipelines the four rows).
    3. dist = |ref - x| via ScalarE Abs with bias=-x (bit-identical to
       the fp32 reference); per-tile DVE chain: gmin = min(dist);
       tie-break exactly like jnp.argmin (lowest ORIGINAL index) via
       cand = idx - 2^17 * (dist == gmin); om = min(cand); answer =
       sum(acc * (cand == om)).  All index arithmetic is exact in fp32.

Monotonicity of fp32 rounding guarantees the global argmin lives in the
gathered window: correctly-rounded subtract is monotone, and near-ties
from rounding cannot occur because nearest-neighbor subtractions are
exact (Sterbenz).  Duplicate values are covered by the stable sort plus
the LPAD/RPAD overlap; exact midpoint ties are covered by the original-
index tie-break.  Bit-for-bit identical to the jax reference.
"""
from contextlib import ExitStack

import numpy as np

import concourse.bass as bass
import concourse.bacc as bacc
import concourse.tile as tile
from concourse import mybir
from concourse._compat import with_exitstack
from concourse.bass_utils import run_bass_kernel_spmd

P = 128
N_CORES = 8
B = 4096
B_CORE = B // N_CORES  # 512
N = 65536
W = 128                 # segment width
NSEG = N // W           # 512 segments
LPAD = 4                # window overlap for duplicate chains (max dup run 3)
RPAD = 4
ROWL = W + LPAD + RPAD  # 136
RPW = 416               # row: refs[0:136] | acc[136:272] | idx[272:408] | pad
N_QT = B_CORE // P      # 4 query tiles per core
SKEW = -131072.0        # -2^17: keeps idx-skew arithmetic exact in fp32

FP32 = mybir.dt.float32
U32 = mybir.dt.uint32


@with_exitstack
def _nn_kernel(ctx: ExitStack, tc: tile.TileContext, xq, bnd, tab, out):
    nc = tc.nc

    persist = ctx.enter_context(tc.tile_pool(name="persist", bufs=1))

    # Constants + a dummy activation that pulls the ~1.3us ACT table load
    # off the critical path before the input DMAs complete.
    bm1 = persist.tile([P, 1], FP32, tag="bm1")
    nc.gpsimd.memset(bm1[:], -1.0)

    # Input DMAs first, all on the Scalar HW-DGE queue: the Sync queue
    # (and any offset-0 half-slice broadcast) lowers into pathological
    # 112-byte packets whose ring drain stalls the teardown barrier.
    # Only the full [P, 0:NSEG] broadcast gets the fat-packet lowering.
    x_sb = persist.tile([P, N_QT], FP32, tag="x_sb")
    nc.scalar.dma_start(out=x_sb[:], in_=xq.rearrange("(p q) -> p q", p=P))
    bnd_b = persist.tile([P, NSEG], FP32, tag="bnd_b")
    nc.scalar.dma_start(out=bnd_b[:], in_=bnd[None, :].to_broadcast([P, NSEG]))
    warm = persist.tile([P, 1], FP32, tag="warm")
    nc.scalar.activation(warm[:], bm1[:], mybir.ActivationFunctionType.Relu)
    neg_x = persist.tile([P, N_QT], FP32, tag="neg_x")
    nc.vector.tensor_scalar_mul(neg_x[:], x_sb[:], -1.0)

    cmp = persist.tile([P, NSEG], FP32, tag="cmp")
    cnt4 = persist.tile([P, N_QT], FP32, tag="cnt4")
    seg4 = persist.tile([P, N_QT], FP32, tag="seg4")
    seg_u4 = persist.tile([P, N_QT], U32, tag="seg_u4")
    gra4 = persist.tile([P, N_QT * RPW], FP32, tag="gra4")
    dist4 = persist.tile([P, N_QT * ROWL], FP32, tag="dist4")

    # --- per query tile: fused is_le+sum on DVE, seg = Relu(cnt-1) ---
    for qt in range(N_QT):
        nc.vector.tensor_scalar(
            cmp[:],
            bnd_b[:],
            x_sb[:, qt : qt + 1],
            None,
            op0=mybir.AluOpType.is_le,
            op1=mybir.AluOpType.add,
            accum_out=cnt4[:, qt : qt + 1],
        )
        nc.scalar.activation(
            seg4[:, qt : qt + 1],
            cnt4[:, qt : qt + 1],
            mybir.ActivationFunctionType.Relu,
            bias=bm1[:, 0:1],
            scale=1.0,
        )
    for qt in range(N_QT):
        nc.vector.tensor_copy(seg_u4[:, qt : qt + 1], seg4[:, qt : qt + 1])
        nc.gpsimd.indirect_dma_start(
            out=gra4[:, qt * RPW : (qt + 1) * RPW],
            out_offset=None,
            in_=tab,
            in_offset=bass.IndirectOffsetOnAxis(ap=seg_u4[:, qt : qt + 1], axis=0),
        )

    # --- phase 2 in slot pairs: |ref-x|, argmin, orig-index tie-break ---
    stage = persist.tile([P, N_QT], FP32, tag="stage")
    for pr in range(N_QT // 2):
        q0 = 2 * pr
        L = 2 * ROWL
        for qt in (q0, q0 + 1):
            nc.scalar.activation(
                dist4[:, qt * ROWL : (qt + 1) * ROWL],
                gra4[:, qt * RPW : qt * RPW + ROWL],
                mybir.ActivationFunctionType.Abs,
                bias=neg_x[:, qt : qt + 1],
                scale=1.0,
            )
        dist2 = dist4[:, q0 * ROWL : q0 * ROWL + L]
        dist2_v = dist2.rearrange("p (q r) -> p q r", q=2)
        gpair_v = gra4[:, q0 * RPW : (q0 + 2) * RPW].rearrange(
            "p (q r) -> p q r", q=2
        )
        gmin2 = persist.tile([P, 2], FP32, tag=f"gmin{pr}")
        nc.vector.tensor_reduce(
            gmin2[:], dist2_v, axis=mybir.AxisListType.X, op=mybir.AluOpType.min
        )
        mask2 = persist.tile([P, L], FP32, tag=f"mask{pr}")
        nc.vector.tensor_tensor(
            out=mask2[:].rearrange("p (q r) -> p q r", q=2),
            in0=dist2_v,
            in1=gmin2[:]
            .rearrange("p (q o) -> p q o", o=1)
            .to_broadcast([P, 2, ROWL]),
            op=mybir.AluOpType.is_equal,
        )
        skew2 = persist.tile([P, L], FP32, tag=f"skew{pr}")
        nc.scalar.activation(
            skew2[:],
            mask2[:],
            mybir.ActivationFunctionType.Copy,
            scale=SKEW,
        )
        cand2 = persist.tile([P, L], FP32, tag=f"cand{pr}")
        nc.vector.tensor_tensor(
            out=cand2[:].rearrange("p (q r) -> p q r", q=2),
            in0=skew2[:].rearrange("p (q r) -> p q r", q=2),
            in1=gpair_v[:, :, 2 * ROWL : 3 * ROWL],
            op=mybir.AluOpType.add,
        )
        om2 = persist.tile([P, 2], FP32, tag=f"om{pr}")
        nc.vector.tensor_reduce(
            om2[:],
            cand2[:].rearrange("p (q r) -> p q r", q=2),
            axis=mybir.AxisListType.X,
            op=mybir.AluOpType.min,
        )
        oh2 = persist.tile([P, L], FP32, tag=f"oh{pr}")
        nc.vector.tensor_tensor(
            out=oh2[:].rearrange("p (q r) -> p q r", q=2),
            in0=cand2[:].rearrange("p (q r) -> p q r", q=2),
            in1=om2[:]
            .rearrange("p (q o) -> p q o", o=1)
            .to_broadcast([P, 2, ROWL]),
            op=mybir.AluOpType.is_equal,
        )
        sel2 = persist.tile([P, L], FP32, tag=f"sel{pr}")
        nc.vector.tensor_tensor(
            out=sel2[:].rearrange("p (q r) -> p q r", q=2),
            in0=oh2[:].rearrange("p (q r) -> p q r", q=2),
            in1=gpair_v[:, :, ROWL : 2 * ROWL],
            op=mybir.AluOpType.mult,
        )
        nc.vector.tensor_reduce(
            stage[:, q0 : q0 + 2],
            sel2[:].rearrange("p (q r) -> p q r", q=2),
            axis=mybir.AxisListType.X,
            op=mybir.AluOpType.add,
        )
    nc.scalar.dma_start(out=out.rearrange("(p q) -> p q", p=P), in_=stage[:])


_CACHED_NC = None


def _build():
    global _CACHED_NC
    if _CACHED_NC is not None:
        return _CACHED_NC
    nc = bacc.Bacc("TRN2", target_bir_lowering=False, debug=False)
    xq = nc.dram_tensor("xq", [B_CORE], FP32, kind="ExternalInput").ap()
    bnd = nc.dram_tensor("bnd", [NSEG], FP32, kind="ExternalInput").ap()
    tab = nc.dram_tensor("tab", [NSEG, RPW], FP32, kind="ExternalInput").ap()
    out = nc.dram_tensor("out", [B_CORE], FP32, kind="ExternalOutput").ap()
    with tile.TileContext(nc) as tc:
        _nn_kernel(tc, xq, bnd, tab, out)
    nc.compile()
    _CACHED_NC = nc
    return nc


def _build_tables(refs, acc):
    order = np.argsort(refs, kind="stable")
    sref = refs[order]
    sacc = acc[order]
    sidx = order.astype(np.float32)
    bnd = np.ascontiguousarray(sref[0::W])
    sref_p = np.concatenate(
        [np.full(LPAD, -1e30, np.float32), sref, np.full(RPAD, 1e30, np.float32)]
    )
    sacc_p = np.concatenate(
        [np.zeros(LPAD, np.float32), sacc, np.zeros(RPAD, np.float32)]
    )
    sidx_p = np.concatenate(
        [np.zeros(LPAD, np.float32), sidx, np.zeros(RPAD, np.float32)]
    )
    gi = (np.arange(NSEG) * W)[:, None] + np.arange(ROWL)[None, :]
    tab = np.zeros((NSEG, RPW), np.float32)
    tab[:, 0:ROWL] = sref_p[gi]
    tab[:, ROWL : 2 * ROWL] = sacc_p[gi]
    tab[:, 2 * ROWL : 3 * ROWL] = sidx_p[gi]
    return bnd, np.ascontiguousarray(tab)


def kernel(x, input_tensor, accuracy_tensor):
    x = np.asarray(x, dtype=np.float32)
    refs = np.asarray(input_tensor, dtype=np.float32)
    acc = np.asarray(accuracy_tensor, dtype=np.float32)

    bnd, tab = _build_tables(refs, acc)
    nc = _build()
    in_maps = [
        {
            "xq": np.ascontiguousarray(x[i * B_CORE : (i + 1) * B_CORE]),
            "bnd": bnd,
            "tab": tab,
        }
        for i in range(N_CORES)
    ]
    res = run_bass_kernel_spmd(nc, in_maps, core_ids=list(range(N_CORES)))
    return np.concatenate([res.results[i]["out"] for i in range(N_CORES)])


# revision 16
# speedup vs baseline: 1.1704x; 1.0277x over previous
"""1-D nearest-neighbor retrieval kernel for Trainium2 (8 NeuronCores).

For each query x[b], finds argmin_n |input_tensor[n] - x[b]| and returns
accuracy_tensor[argmin].  Queries are sharded across the 8 cores (512 each,
4 query tiles of 128 partitions); the index tables are replicated.

Index-based algorithm (host builds a sorted index; device does O(NSEG)
work per query instead of O(N)):
  Host prep: stable-argsort the refs; build
    - bnd[NSEG]: first sorted ref of each 128-wide segment,
    - tab[NSEG, RPW]: per segment a window of the sorted refs (LPAD/RPAD
      overlap into neighboring segments for duplicate chains that
      straddle a boundary), the matching accuracy values, and the
      matching ORIGINAL indices (fp32-exact, < 2^16).
  Device, pipelined per query tile [128 queries in partitions]:
    1. seg = clamp(#{bnd_s <= x} - 1, 0): fused is_le+sum on DVE
       (tensor_scalar with accum_out), Relu(cnt-1) on ScalarE; the
       boundary table is broadcast by two parallel half-DMAs (Sync +
       Scalar HW-DGE queues).
    2. One indirect-DMA row gather per query tile, issued as soon as its
       own seg is ready (GpSimd SWDGE queue pipelines the four rows).
    3. dist = |ref - x| via ScalarE Abs with bias=-x (bit-identical to
       the fp32 reference); per-tile DVE chain: gmin = min(dist);
       tie-break exactly like jnp.argmin (lowest ORIGINAL index) via
       cand = idx - 2^17 * (dist == gmin); om = min(cand); answer =
       sum(acc * (cand == om)).  All index arithmetic is exact in fp32.

Monotonicity of fp32 rounding guarantees the global argmin lives in the
gathered window: correctly-rounded subtract is monotone, and near-ties
from rounding cannot occur because nearest-neighbor subtractions are
exact (Sterbenz).  Duplicate values are covered by the stable sort plus
the LPAD/RPAD overlap; exact midpoint ties are covered by the original-
index tie-break.  Bit-for-bit identical to the jax reference.
"""
from contextlib import ExitStack

import numpy as np

import concourse.bass as bass
import concourse.bacc as bacc
import concourse.tile as tile
from concourse import mybir
from concourse._compat import with_exitstack
from concourse.bass_utils import run_bass_kernel_spmd

P = 128
N_CORES = 8
B = 4096
B_CORE = B // N_CORES  # 512
N = 65536
W = 128                 # segment width
NSEG = N // W           # 512 segments
LPAD = 4                # window overlap for duplicate chains (max dup run 3)
RPAD = 4
ROWL = W + LPAD + RPAD  # 136
RPW = 416               # row: refs[0:136] | acc[136:272] | idx[272:408] | pad
N_QT = B_CORE // P      # 4 query tiles per core
SKEW = -131072.0        # -2^17: keeps idx-skew arithmetic exact in fp32

FP32 = mybir.dt.float32
U32 = mybir.dt.uint32


@with_exitstack
def _nn_kernel(ctx: ExitStack, tc: tile.TileContext, xq, bnd, tab, out):
    nc = tc.nc

    persist = ctx.enter_context(tc.tile_pool(name="persist", bufs=1))

    # Constants + a dummy activation that pulls the ~1.3us ACT table load
    # off the critical path before the input DMAs complete.
    bm1 = persist.tile([P, 1], FP32, tag="bm1")
    nc.gpsimd.memset(bm1[:], -1.0)
    warm = persist.tile([P, 1], FP32, tag="warm")
    nc.scalar.activation(warm[:], bm1[:], mybir.ActivationFunctionType.Relu)

    x_sb = persist.tile([P, N_QT], FP32, tag="x_sb")
    nc.scalar.dma_start(out=x_sb[:], in_=xq.rearrange("(p q) -> p q", p=P))
    # bnd broadcast on the Scalar HW-DGE queue: the Sync queue lowers
    # broadcasts into pathological 112-byte packets whose ring drain
    # stalls the NEFF teardown barrier by ~10us.
    bnd_b = persist.tile([P, NSEG], FP32, tag="bnd_b")
    nc.scalar.dma_start(out=bnd_b[:], in_=bnd[None, :].to_broadcast([P, NSEG]))
    neg_x = persist.tile([P, N_QT], FP32, tag="neg_x")
    nc.vector.tensor_scalar_mul(neg_x[:], x_sb[:], -1.0)

    cmp = persist.tile([P, NSEG], FP32, tag="cmp")
    cnt4 = persist.tile([P, N_QT], FP32, tag="cnt4")
    seg4 = persist.tile([P, N_QT], FP32, tag="seg4")
    seg_u4 = persist.tile([P, N_QT], U32, tag="seg_u4")
    gra4 = persist.tile([P, N_QT * RPW], FP32, tag="gra4")
    dist4 = persist.tile([P, N_QT * ROWL], FP32, tag="dist4")

    # --- per query tile: cnt -> seg -> row gather, pipelined ---
    for qt in range(N_QT):
        nc.vector.tensor_scalar(
            cmp[:],
            bnd_b[:],
            x_sb[:, qt : qt + 1],
            None,
            op0=mybir.AluOpType.is_le,
            op1=mybir.AluOpType.add,
            accum_out=cnt4[:, qt : qt + 1],
        )
        nc.scalar.activation(
            seg4[:, qt : qt + 1],
            cnt4[:, qt : qt + 1],
            mybir.ActivationFunctionType.Relu,
            bias=bm1[:, 0:1],
            scale=1.0,
        )
        nc.vector.tensor_copy(seg_u4[:, qt : qt + 1], seg4[:, qt : qt + 1])
        nc.gpsimd.indirect_dma_start(
            out=gra4[:, qt * RPW : (qt + 1) * RPW],
            out_offset=None,
            in_=tab,
            in_offset=bass.IndirectOffsetOnAxis(ap=seg_u4[:, qt : qt + 1], axis=0),
        )

    # --- per query tile: |ref-x|, argmin with original-index tie-break ---
    stage = persist.tile([P, N_QT], FP32, tag="stage")
    for qt in range(N_QT):
        refs_q = gra4[:, qt * RPW : qt * RPW + ROWL]
        acc_q = gra4[:, qt * RPW + ROWL : qt * RPW + 2 * ROWL]
        idx_q = gra4[:, qt * RPW + 2 * ROWL : qt * RPW + 3 * ROWL]
        dist = dist4[:, qt * ROWL : (qt + 1) * ROWL]
        nc.scalar.activation(
            dist,
            refs_q,
            mybir.ActivationFunctionType.Abs,
            bias=neg_x[:, qt : qt + 1],
            scale=1.0,
        )
        gmin = persist.tile([P, 1], FP32, tag=f"gmin{qt}")
        nc.vector.tensor_reduce(
            gmin[:], dist, axis=mybir.AxisListType.X, op=mybir.AluOpType.min
        )
        mask = persist.tile([P, ROWL], FP32, tag=f"mask{qt}")
        nc.vector.tensor_scalar(
            mask[:], dist, gmin[:, 0:1], None, op0=mybir.AluOpType.is_equal
        )
        skew = persist.tile([P, ROWL], FP32, tag=f"skew{qt}")
        nc.vector.tensor_scalar_mul(skew[:], mask[:], SKEW)
        cand = persist.tile([P, ROWL], FP32, tag=f"cand{qt}")
        nc.vector.tensor_tensor(
            out=cand[:], in0=skew[:], in1=idx_q, op=mybir.AluOpType.add
        )
        om = persist.tile([P, 1], FP32, tag=f"om{qt}")
        nc.vector.tensor_reduce(
            om[:], cand[:], axis=mybir.AxisListType.X, op=mybir.AluOpType.min
        )
        oh = persist.tile([P, ROWL], FP32, tag=f"oh{qt}")
        nc.vector.tensor_scalar(
            oh[:], cand[:], om[:, 0:1], None, op0=mybir.AluOpType.is_equal
        )
        # final dot: sum(oh * acc) fused via is_le?  No -- mult + accum:
        sel = persist.tile([P, ROWL], FP32, tag=f"sel{qt}")
        nc.vector.tensor_tensor(
            out=sel[:], in0=oh[:], in1=acc_q, op=mybir.AluOpType.mult
        )
        nc.vector.tensor_reduce(
            stage[:, qt : qt + 1],
            sel[:],
            axis=mybir.AxisListType.X,
            op=mybir.AluOpType.add,
        )
    nc.scalar.dma_start(out=out.rearrange("(p q) -> p q", p=P), in_=stage[:])


_CACHED_NC = None


def _build():
    global _CACHED_NC
    if _CACHED_NC is not None:
        return _CACHED_NC
    nc = bacc.Bacc("TRN2", target_bir_lowering=False, debug=False)
    xq = nc.dram_tensor("xq", [B_CORE], FP32, kind="ExternalInput").ap()
    bnd = nc.dram_tensor("bnd", [NSEG], FP32, kind="ExternalInput").ap()
    tab = nc.dram_tensor("tab", [NSEG, RPW], FP32, kind="ExternalInput").ap()
    out = nc.dram_tensor("out", [B_CORE], FP32, kind="ExternalOutput").ap()
    with tile.TileContext(nc) as tc:
        _nn_kernel(tc, xq, bnd, tab, out)
    nc.compile()
    _CACHED_NC = nc
    return nc


def _build_tables(refs, acc):
    order = np.argsort(refs, kind="stable")
    sref = refs[order]
    sacc = acc[order]
    sidx = order.astype(np.float32)
    bnd = np.ascontiguousarray(sref[0::W])
    sref_p = np.concatenate(
        [np.full(LPAD, -1e30, np.float32), sref, np.full(RPAD, 1e30, np.float32)]
    )
    sacc_p = np.concatenate(
        [np.zeros(LPAD, np.float32), sacc, np.zeros(RPAD, np.float32)]
    )
    sidx_p = np.concatenate(
        [np.zeros(LPAD, np.float32), sidx, np.zeros(RPAD, np.float32)]
    )
    gi = (np.arange(NSEG) * W)[:, None] + np.arange(ROWL)[None, :]
    tab = np.zeros((NSEG, RPW), np.float32)
    tab[:, 0:ROWL] = sref_p[gi]
    tab[:, ROWL : 2 * ROWL] = sacc_p[gi]
    tab[:, 2 * ROWL : 3 * ROWL] = sidx_p[gi]
    return bnd, np.ascontiguousarray(tab)


def kernel(x, input_tensor, accuracy_tensor):
    x = np.asarray(x, dtype=np.float32)
    refs = np.asarray(input_tensor, dtype=np.float32)
    acc = np.asarray(accuracy_tensor, dtype=np.float32)

    bnd, tab = _build_tables(refs, acc)
    nc = _build()
    in_maps = [
        {
            "xq": np.ascontiguousarray(x[i * B_CORE : (i + 1) * B_CORE]),
            "bnd": bnd,
            "tab": tab,
        }
        for i in range(N_CORES)
    ]
    res = run_bass_kernel_spmd(nc, in_maps, core_ids=list(range(N_CORES)))
    return np.concatenate([res.results[i]["out"] for i in range(N_CORES)])
